# revision 1
# baseline (speedup 1.0000x reference)
"""Trainium2 Bass kernel for BoundaryPointTransformer (gnn_message_passing).

Strategy (8 NeuronCores, data-parallel over points):
  - Each core owns 12500 points (padded to 12544 = 196 blocks x 64 points).
  - Phase A: each core builds a gather table tabKV[100096, 128] =
    [k_tilde | V] rows (device matmuls over the full replicated x), plus a
    point-major q-table for its own points.  BN params are folded into the
    weights/biases on the host (tiny O(C^2) numpy).
  - Phase B: per block of 64 points (1024 point-neighbor pairs):
      * indirect-DMA gather of 1024 table rows (512 B each) + 1024 p rows
      * PE transposes the gathered pair-major rows into channel-major PSUM
      * the attention-weight MLP (BN-folded) runs channel-major on PE/ACT,
        with the two 512-pair super-chunks packed at partition bases {0,32}
      * softmax over the 16 neighbors along the free axis (ACT exp + DVE)
      * weighted value reduction on DVE, PE-transpose back, DMA out rows.
All matmuls use float32r (1 cycle/row on the PE for moving dims >= 256).
"""

import os
import sys

sys.path.insert(0, "/opt/trn_rl_repo")

import numpy as np

import concourse.bass as bass
import concourse.bacc as bacc
import concourse.mybir as mybir
import concourse.tile as tile
from concourse.bass import AP, IndirectOffsetOnAxis

F32 = mybir.dt.float32
F32R = mybir.dt.float32r
I32 = mybir.dt.int32
EPS = 1e-5

# Problem shape (hardcoded per contract).
N = 100000
C = 64
NS = 16
S = 8

NCORES = 8
NPC = N // NCORES            # 12500 points per core
PTS_BLK = 64                 # points per block
PAIRS_BLK = PTS_BLK * NS     # 1024
NBLK = -(-NPC // PTS_BLK)    # 196
NPP = NBLK * PTS_BLK         # 12544 padded points per core
NCHUNK = PAIRS_BLK // 128    # 8 gather chunks per block
NSC = 2                      # super-chunks per block (512 pairs each)
SCW = PAIRS_BLK // NSC       # 512

NPAD = -(-N // 512) * 512    # 100352 padded table rows
NTG = NPAD // 512            # table-build groups (512 pts each)
NQC = NPP // 128             # 98 q-build chunks

_CACHED = {}


def _fold_weights(inp):
    """Fold eval-mode BN into the linear weights. Returns dict of host consts."""
    f = {}
    s1 = inp["w_bn1_g"] / np.sqrt(inp["w_bn1_v"] + EPS)
    c1 = inp["w_bn1_b"] - inp["w_bn1_m"] * s1
    s2 = inp["w_bn2_g"] / np.sqrt(inp["w_bn2_v"] + EPS)
    c2 = inp["w_bn2_b"] - inp["w_bn2_m"] * s2
    s3 = inp["p_bn_g"] / np.sqrt(inp["p_bn_v"] + EPS)
    c3 = inp["p_bn_b"] - inp["p_bn_m"] * s3

    Wk1 = s1[:, None] * inp["Wk"]          # (64, 64)
    Wq1 = s1[:, None] * inp["Wq"]          # (64, 64)
    W1s3 = s3[:, None] * inp["p_w1"]       # (3, 3)

    # tabKV build: out[c, pt] = sum_f WtabC[f, c] * xpT[f, pt]
    # V columns are stored t-major (row r holds channel swap(r)) so the
    # softmax-weight broadcast DMA can use a natural layout.
    swap = (np.arange(64) % 8) * 8 + np.arange(64) // 8
    f["swap"] = swap
    WtabC = np.zeros((68, 128), np.float32)
    WtabC[:64, :64] = Wk1.T
    WtabC[:64, 64:] = inp["Wv"].T[:, swap]
    WtabC[67, 64:] = (inp["bv"] + inp["p_b2"])[swap]
    f["WtabC"] = WtabC

    # q-table build (point-major): row_pt = [-q_tilde (64) | -PW (3) | 0]
    Wq68 = np.zeros((68, 68), np.float32)
    Wq68[:64, :64] = -Wq1.T
    Wq68[64:67, 64:67] = -W1s3.T
    f["Wq68"] = Wq68

    # pair-major 3x3 transform consts: W1s3rep[p, d*3 + d'] = W1s3[d', d]
    f["W1s3rep"] = np.tile(W1s3.T.reshape(1, 9), (128, 1)).astype(np.float32)

    # p passthrough for the table build (row tail = raw p, 4th col zero)
    Wpsel = np.zeros((68, 4), np.float32)
    Wpsel[64, 0] = Wpsel[65, 1] = Wpsel[66, 2] = 1.0
    f["Wpsel"] = Wpsel

    # pLV: rows 0:64 += r3 @ (s1*p_w2).T (logit side), rows 64:128 += r3 @ p_w2.T
    W2LV = np.zeros((3, 128), np.float32)
    W2LV[:, :64] = (s1[:, None] * inp["p_w2"]).T
    W2LV[:, 64:] = inp["p_w2"].T[:, swap]
    f["W2LV"] = W2LV

    W1p = s2[:, None] * inp["w_w1"]        # (8, 64)
    W1pT2 = np.tile(W1p.T, (2, 1))         # (128, 8): both 64-row halves
    f["W1pT2"] = W1pT2
    f["W2T"] = np.ascontiguousarray(inp["w_w2"].T)  # (8, 8)

    # replicate helper for 1/den in t-major value order: out[t*8+s] = in[t]
    f["I8con"] = np.repeat(np.eye(8, dtype=np.float32), 8, axis=1)  # (8, 64)
    # output transpose + channel unpermute: Pperm[r, swap(r)] = 1
    Pperm = np.zeros((64, 64), np.float32)
    Pperm[np.arange(64), swap] = 1.0
    f["Pperm"] = Pperm

    f["biasU"] = (s1 * (inp["bk"] - inp["bq"] + inp["p_b2"]) + c1).astype(
        np.float32
    ).reshape(64, 1)
    bias3 = np.zeros((4, 1), np.float32)
    bias3[:3, 0] = s3 * inp["p_b1"] + c3
    f["bias3"] = bias3
    f["bias1"] = (s2 * inp["w_b1"] + c2).astype(np.float32).reshape(8, 1)
    f["bias2"] = inp["w_b2"].astype(np.float32).reshape(8, 1)
    return f


def _host_prep(inp):
    """Build all device input arrays. Returns (shared dict, per-core list)."""
    f = _fold_weights(inp)
    x = np.asarray(inp["x"], np.float32)
    p = np.asarray(inp["p"], np.float32)
    idx = np.asarray(inp["idx"]).astype(np.int32)

    xpt_full = np.zeros((68, NPAD), np.float32)
    xpt_full[:64, :N] = x.T
    xpt_full[64:67, :N] = p.T
    xpt_full[67, :N] = 1.0

    p_pad = np.zeros((NPAD, 3), np.float32)
    p_pad[:N] = p

    # E2[sc*64 + pt, p] = 1 if (sc*512 + p) // 16 == pt   (pt in [0,64))
    E2 = np.zeros((128, SCW), np.float32)
    for sc in range(NSC):
        pair_pt = (np.arange(SCW) + sc * SCW) // NS
        E2[sc * 64:sc * 64 + 64] = (
            np.arange(PTS_BLK)[:, None] == pair_pt[None, :]
        )

    ident = np.eye(128, dtype=np.float32)

    shared = dict(
        xpt_full=xpt_full, p_pad=p_pad, E2=E2,
        E2b=np.ascontiguousarray(E2[64:128]), ident=ident,
        WtabC=f["WtabC"], Wq68=f["Wq68"], W1s3rep=f["W1s3rep"],
        Wpsel=f["Wpsel"],
        W2LV=f["W2LV"], W1pT2=f["W1pT2"], W2T=f["W2T"],
        I8con=f["I8con"], Pperm=f["Pperm"], biasU=f["biasU"],
        bias3=f["bias3"], bias1=f["bias1"], bias2=f["bias2"],
    )

    per_core = []
    for c in range(NCORES):
        lo = c * NPC
        idx_loc = np.zeros((NPP, NS), np.int32)
        idx_loc[:NPC] = idx[lo:lo + NPC]
        # idxT[p, b*8 + k] = flat_pair[b*1024 + k*128 + p]
        flat = idx_loc.reshape(-1)
        idxT = np.ascontiguousarray(
            flat.reshape(NBLK, NCHUNK, 128).transpose(2, 0, 1).reshape(128, -1)
        )
        xpt_loc = np.zeros((68, NPP), np.float32)
        hi = min(lo + NPP, N)
        xpt_loc[:, : hi - lo] = xpt_full[:, lo:hi]
        per_core.append(dict(idxT=idxT, xpt_loc=xpt_loc))
    return shared, per_core


def _r(ap):
    return ap.bitcast(F32R)


def _build_program():
    """Construct the Bass/Tile program (same for every core)."""
    nc = bacc.Bacc("TRN2", target_bir_lowering=False, debug=False)

    d_xpt_full = nc.dram_tensor("xpt_full", [68, NPAD], F32, kind="ExternalInput")
    d_p_pad = nc.dram_tensor("p_pad", [NPAD, 3], F32, kind="ExternalInput")
    d_E2 = nc.dram_tensor("E2", [128, SCW], F32, kind="ExternalInput")
    d_E2b = nc.dram_tensor("E2b", [64, SCW], F32, kind="ExternalInput")
    d_ident = nc.dram_tensor("ident", [128, 128], F32, kind="ExternalInput")
    d_WtabC = nc.dram_tensor("WtabC", [68, 128], F32, kind="ExternalInput")
    d_Wpsel = nc.dram_tensor("Wpsel", [68, 4], F32, kind="ExternalInput")
    d_Wq68 = nc.dram_tensor("Wq68", [68, 68], F32, kind="ExternalInput")
    d_W1s3rep = nc.dram_tensor("W1s3rep", [128, 9], F32, kind="ExternalInput")
    d_W2LV = nc.dram_tensor("W2LV", [3, 128], F32, kind="ExternalInput")
    d_W1pT2 = nc.dram_tensor("W1pT2", [128, 8], F32, kind="ExternalInput")
    d_W2T = nc.dram_tensor("W2T", [8, 8], F32, kind="ExternalInput")
    d_I8con = nc.dram_tensor("I8con", [8, 64], F32, kind="ExternalInput")
    d_Pperm = nc.dram_tensor("Pperm", [64, 64], F32, kind="ExternalInput")
    d_biasU = nc.dram_tensor("biasU", [64, 1], F32, kind="ExternalInput")
    d_bias3 = nc.dram_tensor("bias3", [4, 1], F32, kind="ExternalInput")
    d_bias1 = nc.dram_tensor("bias1", [8, 1], F32, kind="ExternalInput")
    d_bias2 = nc.dram_tensor("bias2", [8, 1], F32, kind="ExternalInput")
    d_idxT = nc.dram_tensor("idxT", [128, NBLK * NCHUNK], I32, kind="ExternalInput")
    d_xpt_loc = nc.dram_tensor("xpt_loc", [68, NPP], F32, kind="ExternalInput")
    d_out = nc.dram_tensor("out", [NPP, C], F32, kind="ExternalOutput")
    d_tab = nc.dram_tensor("tabKV", [NPAD, 132], F32, kind="Internal")

    RELU = mybir.ActivationFunctionType.Relu
    EXPF = mybir.ActivationFunctionType.Exp

    with tile.TileContext(nc) as tc:
        with tc.tile_pool(name="const", bufs=1) as cp:
            def tile_from_r(dram, name):
                t = cp.tile(list(dram.shape), F32, name=name)
                nc.sync.dma_start(out=_r(t[:, :]), in_=_r(dram.ap()))
                return t

            identS = tile_from_r(d_ident, "identS")
            E2S = tile_from_r(d_E2, "E2S")
            E2bS = tile_from_r(d_E2b, "E2bS")
            WtabCS = tile_from_r(d_WtabC, "WtabCS")
            WpselS = tile_from_r(d_Wpsel, "WpselS")
            Wq68S = tile_from_r(d_Wq68, "Wq68S")
            W1s3repS = tile_from_r(d_W1s3rep, "W1s3repS")
            W2LVS = tile_from_r(d_W2LV, "W2LVS")
            W1pT2S = tile_from_r(d_W1pT2, "W1pT2S")
            W2TS = tile_from_r(d_W2T, "W2TS")
            I8conS = tile_from_r(d_I8con, "I8conS")
            PpermS = tile_from_r(d_Pperm, "PpermS")
            biasUS = cp.tile_from(d_biasU.ap())
            bias3S = cp.tile_from(d_bias3.ap())
            bias1S = cp.tile_from(d_bias1.ap())
            bias2S = cp.tile_from(d_bias2.ap())
            idxTS = cp.tile_from(d_idxT.ap())
            qtab = cp.tile([64, NBLK * 68], F32, name="qtab")

            # ---------------- Phase A1: q-table (point-major) ----------------
            with (
                tc.tile_pool(name="qb", bufs=1) as qb,
                tc.tile_pool(name="qbw", bufs=2) as qbw,
                tc.tile_pool(name="qbp", bufs=2, space="PSUM") as qbp,
            ):
                xptL = qb.tile([68, NPP], F32, name="xptL")
                nc.sync.dma_start(out=_r(xptL[:, :]), in_=_r(d_xpt_loc.ap()))
                for q in range(NBLK):
                    Pq = qbp.tile([64, 68], F32, name="Pq")
                    nc.tensor.matmul(
                        out=Pq[:, :],
                        lhsT=_r(xptL[:, q * 64:(q + 1) * 64]),
                        rhs=_r(Wq68S[:, :]),
                        start=True, stop=True,
                    )
                    if q % 2 == 0:
                        nc.scalar.copy(
                            out=_r(qtab[:, q * 68:(q + 1) * 68]), in_=Pq[:, :]
                        )
                    else:
                        nc.vector.tensor_copy(
                            out=_r(qtab[:, q * 68:(q + 1) * 68]), in_=Pq[:, :]
                        )

            # ---------------- Phase A2: tabKV build ----------------
            with (
                tc.tile_pool(name="tb", bufs=3) as tb,
                tc.tile_pool(name="tbp", bufs=2, space="PSUM") as tbp,
                tc.tile_pool(name="tbp2", bufs=2, space="PSUM") as tbp2,
            ):
                for g in range(int(os.environ.get("K_NTG", NTG))):
                    rhsX = tb.tile([68, 512], F32, name="rhsX")
                    nc.sync.dma_start(
                        out=_r(rhsX[:, :]),
                        in_=_r(d_xpt_full.ap()[:, g * 512:(g + 1) * 512])
                    )
                    Pb = tbp.tile([128, 512], F32, name="Pb")
                    nc.tensor.matmul(
                        out=Pb[:, :], lhsT=_r(WtabCS[:, :]), rhs=_r(rhsX[:, :]),
                        start=True, stop=True,
                    )
                    Pbp = tbp.tile([4, 512], F32, name="Pbp", tag="Pbp")
                    nc.tensor.matmul(
                        out=Pbp[:, :], lhsT=_r(WpselS[:, :]), rhs=_r(rhsX[:, :]),
                        start=True, stop=True,
                    )
                    cbS = tb.tile([128, 512], F32, name="cbS")
                    nc.scalar.copy(out=_r(cbS[:, :256]), in_=Pb[:, :256])
                    nc.vector.tensor_copy(out=_r(cbS[:, 256:]), in_=Pb[:, 256:])
                    cbpS = tb.tile([4, 512], F32, name="cbpS")
                    nc.scalar.copy(out=_r(cbpS[:, :]), in_=Pbp[:, :])
                    tbS = tb.tile([128, 528], F32, name="tbS")
                    for i in range(4):
                        Pt = tbp2.tile([128, 128], F32, name="Pt")
                        nc.tensor.matmul(
                            out=_r(Pt[:, :]),
                            lhsT=_r(cbS[:, i * 128:(i + 1) * 128]),
                            rhs=_r(identS[:, :]),
                            is_transpose=True, start=True, stop=True,
                        )
                        Ptp = tbp2.tile([128, 4], F32, name="Ptp", tag="Ptp")
                        nc.tensor.matmul(
                            out=_r(Ptp[:, :]),
                            lhsT=_r(cbpS[:, i * 128:(i + 1) * 128]),
                            rhs=_r(identS[0:4, 0:4]),
                            is_transpose=True, start=True, stop=True,
                        )
                        dst = tbS[:, i * 132:i * 132 + 128]
                        dstp = tbS[:, i * 132 + 128:i * 132 + 132]
                        if i % 2 == 0:
                            nc.scalar.copy(out=dst, in_=Pt[:, :])
                            nc.vector.tensor_copy(out=dstp, in_=Ptp[:, :])
                        else:
                            nc.vector.tensor_copy(out=dst, in_=Pt[:, :])
                            nc.scalar.copy(out=dstp, in_=Ptp[:, :])
                    # DRAM rows g*512 + i*128 + p  <-  tbS[p, i*132 + c]
                    dram_ap = AP(
                        d_tab.ap().tensor, g * 512 * 132,
                        [[132, 128], [132 * 128, 4], [1, 132]],
                    )
                    src_ap = AP(
                        tbS.tensor, tbS.offset, [[528, 128], [132, 4], [1, 132]]
                    )
                    nc.sync.dma_start(out=dram_ap, in_=src_ap)

            # ---------------- Phase B: main loop ----------------
            with (
                tc.tile_pool(name="mw", bufs=3) as mw,
                tc.tile_pool(name="mw2", bufs=2) as mw2,
                tc.tile_pool(name="pkv", bufs=2, space="PSUM") as pkv_pool,
                tc.tile_pool(name="psA", bufs=2, space="PSUM") as psA,
            ):
                for b in range(int(os.environ.get("K_NBLK", NBLK))):
                    G = mw.tile([128, NCHUNK * 132], F32, name="G")
                    for k in range(NCHUNK):
                        islc = idxTS[:, b * NCHUNK + k:b * NCHUNK + k + 1]
                        nc.gpsimd.indirect_dma_start(
                            out=_r(G[:, k * 132:(k + 1) * 132]), out_offset=None,
                            in_=_r(d_tab.ap()),
                            in_offset=IndirectOffsetOnAxis(ap=islc, axis=0),
                        )

                    if os.environ.get("K_GATHER_ONLY"):
                        gacc = mw2.tile([128, 128], F32, name="gacc", tag="gacc")
                        nc.vector.tensor_copy(out=gacc[:, :], in_=G[:, 0:128])
                        nc.sync.dma_start(
                            out=d_out.ap()[b * PTS_BLK:b * PTS_BLK + 64, :],
                            in_=gacc[0:64, 0:64],
                        )
                        continue
                    # pair-major 3x3 p-transform: G2W[p, 3k+d'] = sum_d W1s3[d',d] G[p,132k+128+d]
                    G2W = mw.tile([128, NCHUNK * 3], F32, name="G2W")
                    tmp3 = mw.tile([128, NCHUNK * 3], F32, name="tmp3")

                    def g2ap(t, d):
                        return AP(t.tensor, t.offset + 128 + d,
                                  [[NCHUNK * 132, 128], [132, NCHUNK], [0, 3]],
                                  ).bitcast(F32R)

                    def wap(d):
                        return AP(W1s3repS.tensor, W1s3repS.offset + 3 * d,
                                  [[9, 128], [0, NCHUNK], [1, 3]]).bitcast(F32R)

                    outap_w = AP(G2W.tensor, G2W.offset,
                                 [[NCHUNK * 3, 128], [3, NCHUNK], [1, 3]],
                                 ).bitcast(F32R)
                    outap_t = AP(tmp3.tensor, tmp3.offset,
                                 [[NCHUNK * 3, 128], [3, NCHUNK], [1, 3]],
                                 ).bitcast(F32R)
                    nc.vector.tensor_tensor(
                        out=outap_w, in0=g2ap(G, 0), in1=wap(0),
                        op=mybir.AluOpType.mult)
                    nc.vector.tensor_tensor(
                        out=outap_t, in0=g2ap(G, 1), in1=wap(1),
                        op=mybir.AluOpType.mult)
                    nc.vector.tensor_tensor(
                        out=outap_w, in0=outap_w, in1=outap_t,
                        op=mybir.AluOpType.add)
                    nc.vector.tensor_tensor(
                        out=outap_t, in0=g2ap(G, 2), in1=wap(2),
                        op=mybir.AluOpType.mult)
                    nc.vector.tensor_tensor(
                        out=outap_w, in0=outap_w, in1=outap_t,
                        op=mybir.AluOpType.add)

                    # q-block slices (point-major q-table, base partition 0)
                    qcol = b * 68
                    qx = qtab[:, qcol:qcol + 64]
                    qpw4 = qtab[:, qcol + 64:qcol + 68]

                    # kv transposes into PSUM
                    Pkv = pkv_pool.tile([128, PAIRS_BLK], F32, name="Pkv")
                    for k in range(NCHUNK):
                        nc.tensor.matmul(
                            out=_r(Pkv[:, k * 128:(k + 1) * 128]),
                            lhsT=_r(G[:, k * 132:k * 132 + 128]),
                            rhs=_r(identS[:, :]),
                            is_transpose=True, start=(k % 4 == 0), stop=False,
                            skip_group_check=True,
                        )

                    # z3 = -PW_i expansion (rows 0:4) + PW_s transposes (rows 0:3)
                    P2z = psA.tile([4, PAIRS_BLK], F32, name="P2z", tag="psA")
                    for sc in range(NSC):
                        e2sc = E2S[0:64, :] if sc == 0 else E2bS[:, :]
                        nc.tensor.matmul(
                            out=P2z[:, sc * SCW:(sc + 1) * SCW],
                            lhsT=_r(qpw4), rhs=_r(e2sc),
                            start=True, stop=False, skip_group_check=True,
                        )
                    for k in range(NCHUNK):
                        nc.tensor.matmul(
                            out=_r(P2z[0:3, k * 128:(k + 1) * 128]),
                            lhsT=_r(G2W[:, 3 * k:3 * k + 3]),
                            rhs=_r(identS[:, :]),
                            is_transpose=True, start=False, stop=True,
                            skip_group_check=True,
                        )
                    r3S = mw.tile([4, PAIRS_BLK], F32, name="r3S")
                    nc.scalar.activation(
                        out=_r(r3S[:, :]), in_=P2z[:, :], func=RELU,
                        bias=bias3S[:, :],
                    )

                    # logit accumulation: -q_i expansion + p_r(L|V)
                    for sc in range(NSC):
                        e2sc = E2S[0:64, :] if sc == 0 else E2bS[:, :]
                        nc.tensor.matmul(
                            out=Pkv[0:64, sc * SCW:(sc + 1) * SCW],
                            lhsT=_r(qx), rhs=_r(e2sc),
                            start=False, stop=False, skip_group_check=True,
                        )
                        nc.tensor.matmul(
                            out=Pkv[:, sc * SCW:(sc + 1) * SCW],
                            lhsT=_r(W2LVS[:, :]),
                            rhs=_r(r3S[0:3, sc * SCW:(sc + 1) * SCW]),
                            start=False, stop=True, skip_group_check=True,
                        )

                    # u2[(sc,c), p] = relu(u[c, sc*512+p] + biasU)
                    u2 = mw.tile([128, SCW], F32, name="u2")
                    nc.scalar.activation(
                        out=_r(u2[0:64, :]), in_=Pkv[0:64, 0:SCW],
                        func=RELU, bias=biasUS[:, :],
                    )
                    nc.scalar.activation(
                        out=_r(u2[64:128, :]), in_=Pkv[0:64, SCW:2 * SCW],
                        func=RELU, bias=biasUS[:, :],
                    )

                    Py1 = psA.tile([8, PAIRS_BLK], F32, name="Py1", tag="psA")
                    for sc in range(NSC):
                        nc.tensor.matmul(
                            out=Py1[:, sc * SCW:(sc + 1) * SCW],
                            lhsT=_r(W1pT2S[64 * sc:64 * sc + 64, :]),
                            rhs=_r(u2[64 * sc:64 * sc + 64, :]),
                            start=True, stop=True, skip_group_check=True,
                        )
                    y1S = mw2.tile([8, PAIRS_BLK], F32, name="y1S")
                    nc.scalar.activation(
                        out=_r(y1S[:, :]), in_=Py1[:, :], func=RELU,
                        bias=bias1S[:, :],
                    )
                    PL = psA.tile([8, PAIRS_BLK], F32, name="PL", tag="psA")
                    for sc in range(NSC):
                        nc.tensor.matmul(
                            out=PL[:, sc * SCW:(sc + 1) * SCW],
                            lhsT=_r(W2TS[:, :]),
                            rhs=_r(y1S[:, sc * SCW:(sc + 1) * SCW]),
                            start=True, stop=True, skip_group_check=True,
                        )
                    eS = mw2.tile([8, PAIRS_BLK], F32, name="eS")
                    nc.scalar.activation(
                        out=_r(eS[:, :]), in_=PL[:, :], func=EXPF,
                        bias=bias2S[:, :],
                    )
                    denS = mw2.tile([8, PTS_BLK], F32, name="denS")
                    nc.vector.tensor_reduce(
                        out=denS[:, :],
                        in_=AP(eS.tensor, eS.offset,
                               [[PAIRS_BLK, 8], [NS, PTS_BLK], [1, NS]]),
                        axis=mybir.AxisListType.X, op=mybir.AluOpType.add,
                    )
                    rdenS = mw2.tile([8, PTS_BLK], F32, name="rdenS")
                    with nc.allow_low_precision(reason="f32r bitcast, same width"):
                        nc.vector.reciprocal(out=_r(rdenS[:, :]), in_=denS[:, :])

                    # replicate raw exp weights: erepS[t*8+s, p] = eS[t, p]
                    erepS = mw.tile([64, PAIRS_BLK], F32, name="erepS")
                    in_rep = AP(eS.tensor, eS.offset,
                                [[PAIRS_BLK, 8], [0, 8], [1, PAIRS_BLK]])
                    nc.sync.dma_start(out=erepS[:, :], in_=in_rep)

                    vwS = mw.tile([64, PAIRS_BLK], F32, name="vwS")
                    nc.vector.tensor_tensor(
                        out=vwS[:, :], in0=Pkv[64:128, :], in1=erepS[:, :],
                        op=mybir.AluOpType.mult,
                    )
                    numS = mw2.tile([64, PTS_BLK], F32, name="numS")
                    nc.vector.tensor_reduce(
                        out=numS[:, :],
                        in_=AP(vwS.tensor, vwS.offset,
                               [[PAIRS_BLK, 64], [NS, PTS_BLK], [1, NS]]),
                        axis=mybir.AxisListType.X, op=mybir.AluOpType.add,
                    )
                    # replicate 1/den to the t-major value rows
                    Pden = psA.tile([64, PTS_BLK], F32, name="Pden", tag="psA")
                    nc.tensor.matmul(
                        out=Pden[:, :], lhsT=_r(I8conS[:, :]),
                        rhs=_r(rdenS[:, :]),
                        start=True, stop=True, skip_group_check=True,
                    )
                    sm = mw2.tile([64, PTS_BLK], F32, name="sm")
                    nc.vector.tensor_tensor(
                        out=_r(sm[:, :]), in0=numS[:, :], in1=Pden[:, :],
                        op=mybir.AluOpType.mult,
                    )
                    # transpose back to point-major + un-permute value channels
                    Pout = psA.tile([64, 64], F32, name="Pout", tag="psA")
                    nc.tensor.matmul(
                        out=_r(Pout[:, :]), lhsT=_r(sm[:, :]),
                        rhs=_r(PpermS[:, :]),
                        is_transpose=True, start=True, stop=True,
                        skip_group_check=True,
                    )
                    outS = mw2.tile([64, 64], F32, name="outS")
                    nc.scalar.copy(out=outS[:, :], in_=Pout[:, :])
                    nc.sync.dma_start(
                        out=d_out.ap()[b * PTS_BLK:(b + 1) * PTS_BLK, :],
                        in_=outS[:, :],
                    )

    nc.compile()
    return nc


def kernel(**inputs):
    from concourse.bass_utils import run_bass_kernel_spmd

    shared, per_core = _host_prep(inputs)

    if "nc" not in _CACHED:
        _CACHED["nc"] = _build_program()
    nc = _CACHED["nc"]

    in_maps = []
    for c in range(NCORES):
        m = dict(shared)
        m.update(per_core[c])
        in_maps.append(m)

    res = run_bass_kernel_spmd(nc, in_maps, core_ids=list(range(NCORES)))
    out = np.empty((N, C), np.float32)
    for c in range(NCORES):
        out[c * NPC:(c + 1) * NPC] = res.results[c]["out"][:NPC]
    return out

